# revision 7
# baseline (speedup 1.0000x reference)
"""Trainium2 Bass kernel for nn_ModalDecoder (embedding_lookup).

Reference computation:
    w  = out_projection_table[idx].reshape(B, F, D, O)      # [B,F,D,O]
    b  = feature_bias_table[idx]                            # [B,F,D]
    xb = x[:, :, None, :] + b[:, None, :, :]                # [B,N,F,D]
    out = einsum('bnfd,bfdo->bnfo', xb, w)                  # [B,N,F,O]

Factorization (avoids the 128MB [B,N,F,D] intermediate):
    out[b, n, f, :] = x[b, n, :] @ W[b, f] + (bias[b, f] @ W[b, f])
The bias term is a per-(b,f) length-O vector, broadcast over n; it is
precomputed on host (B*F*D*O MACs, tiny) and added on-device per PSUM tile
via tensor_scalar_add (per-partition scalar).

Sharding: 8 cores = 4 values of b x 2 halves of N. Per core:
    y[fo, n] = Wpack[d, fo].T @ xT[d, n] + cvec[fo]
with Wpack = [D, F*O] (host-gathered tables packed side by side), xT the
transposed x half, both bf16 (PSUM accumulates fp32). y is [F*O, NH] bf16
(host upcasts; bf16 output keeps rel err ~3e-3 vs the 2e-2 gate and halves
store traffic to 1MB/core).

Device kernel is raw Bass (no TileContext -> avoids Tile's expensive
kernel-tail drain + barrier butterfly). Manual semaphores, one cumulative
completion count per HWDGE ring (ring completion is FIFO per issuing
engine, so one counting sem gates every chunk on that ring). Loads are
split across BOTH HWDGE rings (sync + scalar) because descriptor
generation costs ~600ns of sequencer time per dma_start - serializing 7
loads on one ring made load issue, not bytes, the bottleneck. cv rides the
gpsimd SWDGE queue, fully off the critical path. xT is laid out
n-half-major so the first matmul group needs only 384KB landed (xt half +
wp group 0) instead of 640KB. Matmuls are N=256; the PE is pre-warmed with
short N=128 dummy matmuls (cold issue rate ~107ns each) so the HAM clock
gate flips to 2.4GHz right as real data lands, without a long warmup
matmul blocking the first real group. Stores (bf16, 128KB per group)
alternate rings; the final group is stored as two 64KB halves, one per
ring, so the last transfer after the last matmul is short. No explicit
end-of-kernel completion waits or semaphore clears: the NEFF's own
epilogue drains in-flight DMAs and re-zeroes every semaphore, which also
overlaps the last store transfer.

Per-core HBM traffic: 0.5MB xT + 1MB Wpack + 1MB out (memory-bound).
"""

import numpy as np
import ml_dtypes

B, N, D, O, F, V = 4, 1024, 512, 64, 16, 64
NH = N // 2            # 512 rows of x per core
FO = F * O             # 1024 packed output columns
KT = D // 128          # 4 contraction chunks
ST = FO // 128         # 8 output-partition chunks
NU = 2                 # n-half split within a core (256 cols per matmul)
NQ = NH // NU          # 256
N_WARM = 22            # short N=128 PE warmup matmuls during load phase

_cache: dict = {}


def _build_program(with_clears=True):
    # with_clears=True is the real (HW) program. The False variant is for
    # CoreSim validation: it enables the race detector and memsets the
    # warmup scratch (CoreSim rejects reads of uninitialized SBUF; on HW
    # the warmup matmul inputs are garbage by design and never observed).
    import concourse.bass as bass
    import concourse.mybir as mybir

    bf16 = mybir.dt.bfloat16
    f32 = mybir.dt.float32

    nc = bass.Bass(
        "TRN2",
        target_bir_lowering=False,
        debug=False,
        num_devices=8,
        detect_race_conditions=not with_clears,
    )

    # xt columns: u*1024 + k*256 + n  (u = n-half, k = contraction chunk)
    xt_d = nc.dram_tensor("xt", [128, NU * KT * NQ], bf16, kind="ExternalInput")
    # wp columns: s*512 + k*128 + c  (s = output-partition chunk)
    wp_d = nc.dram_tensor("wp", [128, ST * KT * 128], bf16, kind="ExternalInput")
    cv_d = nc.dram_tensor("cv", [128, ST], f32, kind="ExternalInput")
    y_d = nc.dram_tensor("y", [FO, NH], bf16, kind="ExternalOutput")

    yv = y_d.ap().rearrange("(g p) n -> p g n", p=128)  # [128, ST, NH]

    with (
        nc.sbuf_tensor("xt_sb", [128, NU * KT * NQ], bf16) as xt_sb,
        nc.sbuf_tensor("wp_sb", [128, ST * KT * 128], bf16) as wp_sb,
        nc.sbuf_tensor("cv_sb", [128, ST], f32) as cv_sb,
        nc.sbuf_tensor("out_sb", [128, ST, NH], bf16) as out_sb,
        nc.sbuf_tensor("scr_sb", [128, 128], bf16) as scr_sb,
        nc.psum_tensor([128, ST, NH], f32) as ps,
        nc.semaphore("s_xt0") as s_xt0,
        nc.semaphore("s_xt1") as s_xt1,
        nc.semaphore("s_wp45") as s_wp45,
        nc.semaphore("s_wp67") as s_wp67,
        nc.semaphore("s_wp0") as s_wp0,
        nc.semaphore("s_wp1") as s_wp1,
        nc.semaphore("s_wp23") as s_wp23,
        nc.semaphore("s_cv") as s_cv,
        nc.semaphore("s_mm") as s_mm,             # one inc per (s,u) group
        nc.semaphore("s_dve_sync") as s_dve_sync,
        nc.semaphore("s_dve_act") as s_dve_act,
        nc.semaphore("s_st_sync") as s_st_sync,   # store completion (never waited)
        nc.semaphore("s_st_act") as s_st_act,
        nc.semaphore("s_ws") as s_ws,
        nc.Block() as block,
    ):

        @block.sync
        def _(sync):
            # Loads split across both HWDGE rings: issue cost is ~600ns of
            # sequencer time per dma_start, so two rings halve time-to-Nth-
            # chunk. Ring completion is FIFO -> one cumulative sem per ring.
            sync.dma_start(xt_sb[:, 0:1024], xt_d.ap()[:, 0:1024]).then_inc(s_xt0, 16)
            sync.dma_start(xt_sb[:, 1024:2048], xt_d.ap()[:, 1024:2048]).then_inc(
                s_xt1, 16
            )
            sync.dma_start(wp_sb[:, 2048:3072], wp_d.ap()[:, 2048:3072]).then_inc(
                s_wp45, 16
            )
            sync.dma_start(wp_sb[:, 3072:4096], wp_d.ap()[:, 3072:4096]).then_inc(
                s_wp67, 16
            )
            for j, s in enumerate((0, 2, 4, 6)):
                sync.wait_ge(s_dve_sync, j + 1)
                sync.dma_start(yv[:, s, :], out_sb[:, s, :]).then_inc(s_st_sync, 16)
            sync.wait_ge(s_dve_sync, 5)
            sync.dma_start(yv[:, 7, 256:512], out_sb[:, 7, 256:512]).then_inc(
                s_st_sync, 16
            )
            # No final completion wait: the framework epilogue's DRAIN retires
            # in-flight DMAs, and its semaphore sweep re-zeroes every sem.

        @block.scalar
        def _(scalar):
            scalar.dma_start(wp_sb[:, 0:512], wp_d.ap()[:, 0:512]).then_inc(s_wp0, 16)
            scalar.dma_start(wp_sb[:, 512:1024], wp_d.ap()[:, 512:1024]).then_inc(
                s_wp1, 16
            )
            scalar.dma_start(wp_sb[:, 1024:2048], wp_d.ap()[:, 1024:2048]).then_inc(
                s_wp23, 16
            )
            for j, s in enumerate((1, 3, 5)):
                scalar.wait_ge(s_dve_act, j + 1)
                scalar.dma_start(yv[:, s, :], out_sb[:, s, :]).then_inc(s_st_act, 16)
            scalar.wait_ge(s_dve_act, 4)
            scalar.dma_start(yv[:, 7, 0:256], out_sb[:, 7, 0:256]).then_inc(
                s_st_act, 16
            )

        @block.gpsimd
        def _(gpsimd):
            # Tiny bias-vector load on the SWDGE queue - off both HWDGE rings.
            gpsimd.dma_start(cv_sb[:], cv_d.ap()).then_inc(s_cv, 16)

        @block.tensor
        def _(tensor):
            # Warm the PE HAM clock gate while loads are in flight. scr_sb is
            # never written on HW (garbage is fine - the warmup PSUM region is
            # overwritten with start=True by group (7,0) before any read); the
            # sim variant memsets it because CoreSim rejects uninit reads.
            # N=128 warmups issue every ~107ns cold, so the PE can pivot to
            # real work almost immediately when the first gate opens.
            if not with_clears:
                tensor.wait_ge(s_ws, 1)
            for _ in range(N_WARM):
                nc.tensor.matmul(
                    ps[:, 7, 0:128],
                    scr_sb[:],
                    scr_sb[:],
                    start=True,
                    stop=True,
                )
            for s in range(ST):
                if s == 0:
                    tensor.wait_ge(s_xt0, 16)
                    tensor.wait_ge(s_wp0, 16)
                elif s == 1:
                    tensor.wait_ge(s_wp1, 16)
                elif s == 2:
                    tensor.wait_ge(s_wp23, 16)
                elif s == 4:
                    tensor.wait_ge(s_wp45, 16)
                elif s == 6:
                    tensor.wait_ge(s_wp67, 16)
                for u in range(NU):
                    if s == 0 and u == 1:
                        tensor.wait_ge(s_xt1, 16)
                    for k in range(KT):
                        inst = nc.tensor.matmul(
                            ps[:, s, u * NQ:(u + 1) * NQ],
                            wp_sb[:, s * 512 + k * 128:s * 512 + (k + 1) * 128],
                            xt_sb[:, u * 1024 + k * NQ:u * 1024 + (k + 1) * NQ],
                            start=(k == 0),
                            stop=(k == KT - 1),
                        )
                        if k == KT - 1:
                            inst.then_inc(s_mm, 1)

        @block.vector
        def _(vector):
            if not with_clears:
                vector.memset(scr_sb[:], 0).then_inc(s_ws, 1)
            vector.wait_ge(s_cv, 16)  # cv loaded
            # inc map: (s,1) for s=0,2,4,6 -> s_dve_sync 1..4; (s,1) for
            # s=1,3,5 -> s_dve_act 1..3; (7,0) -> s_dve_act 4; (7,1) ->
            # s_dve_sync 5. Stores per group s wait on the matching count.
            for s in range(ST):
                # Both halves of bank s must close before reading either
                # (CoreSim tracks accumulation groups at bank granularity).
                vector.wait_ge(s_mm, 2 * s + 2)
                for u in range(NU):
                    inst = nc.vector.tensor_scalar_add(
                        out_sb[:, s, u * NQ:(u + 1) * NQ],
                        ps[:, s, u * NQ:(u + 1) * NQ],
                        cv_sb[:, s:s + 1],
                    )
                    if s == 7:
                        inst.then_inc(s_dve_act if u == 0 else s_dve_sync, 1)
                    elif u == 1:
                        inst.then_inc(s_dve_sync if s % 2 == 0 else s_dve_act, 1)

    return nc


def _get_program():
    nc = _cache.get("nc")
    if nc is None:
        nc = _build_program()
        _cache["nc"] = nc
    return nc


def _prep_in_maps(x, idx, fbt, opt):
    bf = ml_dtypes.bfloat16
    in_maps = []
    for b in range(B):
        w = opt[idx[b]].reshape(F, D, O)                     # [F,D,O] f32
        wpack = w.transpose(1, 0, 2).reshape(KT, 128, ST, 128)  # [k,p,s,c]
        wp_host = np.ascontiguousarray(
            wpack.transpose(1, 2, 0, 3).reshape(128, KT * FO)
        ).astype(bf)                                         # [p, s*512+k*128+c]
        bias = fbt[idx[b]]                                   # [F,D]
        cvec = np.einsum("fd,fdo->fo", bias, w).reshape(FO).astype(np.float32)
        cv = np.ascontiguousarray(cvec.reshape(ST, 128).T)   # [128, ST]
        for h in range(2):
            xtT = x[b, h * NH:(h + 1) * NH, :].T             # [D, NH]
            # columns u*1024 + k*256 + n (n-half-major)
            xt_host = np.ascontiguousarray(
                xtT.reshape(KT, 128, NU, NQ).transpose(1, 2, 0, 3).reshape(
                    128, NU * KT * NQ
                )
            ).astype(bf)
            in_maps.append({"xt": xt_host, "wp": wp_host, "cv": cv})
    return in_maps


def _assemble(results):
    out = np.empty((B, N, F, O), dtype=np.float32)
    for c in range(8):
        b, h = divmod(c, 2)
        y = np.asarray(results[c]["y"]).astype(np.float32)   # [FO, NH] bf16
        out[b, h * NH:(h + 1) * NH] = y.reshape(F, O, NH).transpose(2, 0, 1)
    return out


def _run(x, idx, feature_bias_table, out_projection_table, **run_kwargs):
    from concourse.bass_utils import run_bass_kernel_spmd

    x = np.asarray(x, dtype=np.float32)
    idx = np.asarray(idx).astype(np.int64)
    fbt = np.asarray(feature_bias_table, dtype=np.float32)
    opt = np.asarray(out_projection_table, dtype=np.float32)

    nc = _get_program()
    in_maps = _prep_in_maps(x, idx, fbt, opt)
    res = run_bass_kernel_spmd(nc, in_maps, core_ids=list(range(8)), **run_kwargs)
    return _assemble(res.results), res


def kernel(x, idx, feature_bias_table, out_projection_table):
    out, _ = _run(x, idx, feature_bias_table, out_projection_table)
    return out


# revision 8
# speedup vs baseline: 1.2478x; 1.2478x over previous
"""Trainium2 Bass kernel for nn_ModalDecoder (embedding_lookup).

Reference computation:
    w  = out_projection_table[idx].reshape(B, F, D, O)      # [B,F,D,O]
    b  = feature_bias_table[idx]                            # [B,F,D]
    xb = x[:, :, None, :] + b[:, None, :, :]                # [B,N,F,D]
    out = einsum('bnfd,bfdo->bnfo', xb, w)                  # [B,N,F,O]

Factorization (avoids the 128MB [B,N,F,D] intermediate):
    out[b, n, f, :] = x[b, n, :] @ W[b, f] + (bias[b, f] @ W[b, f])
The bias term is a per-(b,f) length-O vector, broadcast over n; it is
precomputed on host (tiny) and added on-device per PSUM tile via
tensor_scalar_add (per-partition scalar), which doubles as the mandatory
PSUM->SBUF copy (DMA cannot read PSUM).

Sharding: 8 cores = 4 values of b x 2 halves of N. Per core:
    y[fo, n] = Wpack[d, fo].T @ xT[d, n] + cvec[fo]
with Wpack = [D, F*O] (host-gathered tables packed side by side), xT the
transposed x half, both bf16 (PSUM accumulates fp32). y is [F*O, NH] bf16
(host upcasts; bf16 output keeps rel err ~3e-3 vs the 2e-2 gate and halves
store traffic to 1MB/core).

Schedule (from perfetto analysis; ~7us of framework preamble and ~1.7us of
epilogue bracket everything):
  * All loads go on the sync HWDGE ring, ordered by when the PE needs them:
    wp0, xt_n0, xt_n1, wp1, wp23, wp45, wp67. One ring gets the full
    ~360GB/s; a previous two-ring split halved the effective rate of the
    gating chunks and starved the PE. The first matmul group needs only
    wp0+xt_n0 = 384KB on the wire.
  * xt keeps the k-major layout; the two xt DMA chunks are the n-halves
    (4x512B strided runs per partition - exactly at the 512B line-rate
    boundary). Group 0 runs as two N=256 half-groups so it can start on
    xt_n0 alone; groups 1-7 run as N=512 (warm pace 216ns vs 2x131ns -
    LDWEIGHTS is not fully hidden at N=256).
  * The PE is pre-warmed with N=128 dummy matmuls (cold issue ~128ns each)
    sized to end right as wp0+xt_n0 land, so the HAM clock gate flips to
    2.4GHz during the first real (cold) groups with no idle gap.
  * cv + the early stores ride the scalar HWDGE ring (stores s0-s3 and the
    n0-half of s7); the sync ring takes s4-s6 and the n1-half of s7 after
    its loads. Splitting s7's store in two 64KB halves, one per ring,
    shortens the post-last-matmul tail.
  * DVE waits for both half-groups of a bank before its two 256-wide adds
    (CoreSim tracks accumulation at bank granularity; the halves also feed
    the two store rings independently).
  * No end-of-kernel waits or sem clears: the NEFF epilogue drains DMAs
    and re-zeroes every semaphore, overlapping the last store transfer.

Per-core HBM traffic: 0.5MB xT + 1MB Wpack + 1MB out (memory-bound).
"""

import numpy as np
import ml_dtypes

B, N, D, O, F, V = 4, 1024, 512, 64, 16, 64
NH = N // 2            # 512 rows of x per core
FO = F * O             # 1024 packed output columns
KT = D // 128          # 4 contraction chunks
ST = FO // 128         # 8 output-partition chunks
NQ = NH // 2           # 256 (n-half within a core, group-0 split)
N_WARM = 18            # short N=128 PE warmup matmuls during load phase

_cache: dict = {}


def _build_program(with_clears=True):
    # with_clears=True is the real (HW) program. The False variant is for
    # CoreSim validation: it enables the race detector and memsets the
    # warmup scratch (CoreSim rejects reads of uninitialized SBUF; on HW
    # the warmup matmul inputs are garbage by design and never observed).
    import concourse.bass as bass
    import concourse.mybir as mybir

    bf16 = mybir.dt.bfloat16
    f32 = mybir.dt.float32

    nc = bass.Bass(
        "TRN2",
        target_bir_lowering=False,
        debug=False,
        num_devices=8,
        detect_race_conditions=not with_clears,
    )

    # xt columns: k*512 + n  (k-major; n-halves are 256-wide sub-slices)
    xt_d = nc.dram_tensor("xt", [128, KT, 2, NQ], bf16, kind="ExternalInput")
    # wp columns: s*512 + k*128 + c  (s = output-partition chunk)
    wp_d = nc.dram_tensor("wp", [128, ST * KT * 128], bf16, kind="ExternalInput")
    cv_d = nc.dram_tensor("cv", [128, ST], f32, kind="ExternalInput")
    y_d = nc.dram_tensor("y", [FO, NH], bf16, kind="ExternalOutput")

    yv = y_d.ap().rearrange("(g p) n -> p g n", p=128)  # [128, ST, NH]

    with (
        nc.sbuf_tensor("xt_sb", [128, KT, 2, NQ], bf16) as xt_sb,
        nc.sbuf_tensor("wp_sb", [128, ST * KT * 128], bf16) as wp_sb,
        nc.sbuf_tensor("cv_sb", [128, ST], f32) as cv_sb,
        nc.sbuf_tensor("out_sb", [128, ST, NH], bf16) as out_sb,
        nc.sbuf_tensor("scr_sb", [128, 128], bf16) as scr_sb,
        nc.psum_tensor([128, ST, NH], f32) as ps,
        nc.semaphore("s_wp0") as s_wp0,
        nc.semaphore("s_xt0") as s_xt0,
        nc.semaphore("s_xt1") as s_xt1,
        nc.semaphore("s_wp1") as s_wp1,
        nc.semaphore("s_wp23") as s_wp23,
        nc.semaphore("s_wp45") as s_wp45,
        nc.semaphore("s_wp67") as s_wp67,
        nc.semaphore("s_cv") as s_cv,
        nc.semaphore("s_mm") as s_mm,
        nc.semaphore("s_dve_sync") as s_dve_sync,
        nc.semaphore("s_dve_act") as s_dve_act,
        nc.semaphore("s_st_sync") as s_st_sync,   # store completion (never waited)
        nc.semaphore("s_st_act") as s_st_act,
        nc.semaphore("s_ws") as s_ws,
        nc.Block() as block,
    ):

        @block.sync
        def _(sync):
            # All loads on one ring, in PE-need order (ring keeps the full
            # SDMA rate; the gating chunks wp0+xt_n0 total 384KB).
            sync.dma_start(wp_sb[:, 0:512], wp_d.ap()[:, 0:512]).then_inc(s_wp0, 16)
            sync.dma_start(xt_sb[:, :, 0, :], xt_d.ap()[:, :, 0, :]).then_inc(
                s_xt0, 16
            )
            sync.dma_start(xt_sb[:, :, 1, :], xt_d.ap()[:, :, 1, :]).then_inc(
                s_xt1, 16
            )
            sync.dma_start(wp_sb[:, 512:1024], wp_d.ap()[:, 512:1024]).then_inc(
                s_wp1, 16
            )
            sync.dma_start(wp_sb[:, 1024:2048], wp_d.ap()[:, 1024:2048]).then_inc(
                s_wp23, 16
            )
            sync.dma_start(wp_sb[:, 2048:3072], wp_d.ap()[:, 2048:3072]).then_inc(
                s_wp45, 16
            )
            sync.dma_start(wp_sb[:, 3072:4096], wp_d.ap()[:, 3072:4096]).then_inc(
                s_wp67, 16
            )
            for j, s in enumerate((4, 5, 6)):
                sync.wait_ge(s_dve_sync, j + 1)
                sync.dma_start(yv[:, s, :], out_sb[:, s, :]).then_inc(s_st_sync, 16)
            sync.wait_ge(s_dve_sync, 4)
            sync.dma_start(yv[:, 7, 256:512], out_sb[:, 7, 256:512]).then_inc(
                s_st_sync, 16
            )
            # No final completion wait: the framework epilogue's DRAIN retires
            # in-flight DMAs, and its semaphore sweep re-zeroes every sem.

        @block.scalar
        def _(scalar):
            # cv also primes this ring's DMA path before the stores.
            scalar.dma_start(cv_sb[:], cv_d.ap()).then_inc(s_cv, 16)
            for j, s in enumerate((0, 1, 2, 3)):
                scalar.wait_ge(s_dve_act, j + 1)
                scalar.dma_start(yv[:, s, :], out_sb[:, s, :]).then_inc(s_st_act, 16)
            scalar.wait_ge(s_dve_act, 5)
            scalar.dma_start(yv[:, 7, 0:256], out_sb[:, 7, 0:256]).then_inc(
                s_st_act, 16
            )

        @block.tensor
        def _(tensor):
            # Warm the PE HAM clock gate while loads are in flight. scr_sb is
            # never written on HW (garbage is fine - the warmup PSUM region is
            # overwritten with start=True by group 7 before any read); the
            # sim variant memsets it because CoreSim rejects uninit reads.
            if not with_clears:
                tensor.wait_ge(s_ws, 1)
            for _ in range(N_WARM):
                nc.tensor.matmul(
                    ps[:, 7, 0:128],
                    scr_sb[:],
                    scr_sb[:],
                    start=True,
                    stop=True,
                )
            # Group 0: two N=256 half-groups, gated on xt n-halves.
            tensor.wait_ge(s_wp0, 16)
            tensor.wait_ge(s_xt0, 16)
            for u in range(2):
                if u == 1:
                    tensor.wait_ge(s_xt1, 16)
                for k in range(KT):
                    inst = nc.tensor.matmul(
                        ps[:, 0, u * NQ:(u + 1) * NQ],
                        wp_sb[:, k * 128:(k + 1) * 128],
                        xt_sb[:, k, u, :],
                        start=(k == 0),
                        stop=(k == KT - 1),
                    )
                    if k == KT - 1:
                        inst.then_inc(s_mm, 1)
            # Groups 1-7: full N=512 matmuls.
            for s in range(1, ST):
                if s == 1:
                    tensor.wait_ge(s_wp1, 16)
                elif s == 2:
                    tensor.wait_ge(s_wp23, 16)
                elif s == 4:
                    tensor.wait_ge(s_wp45, 16)
                elif s == 6:
                    tensor.wait_ge(s_wp67, 16)
                for k in range(KT):
                    inst = nc.tensor.matmul(
                        ps[:, s, :],
                        wp_sb[:, s * 512 + k * 128:s * 512 + (k + 1) * 128],
                        xt_sb[:, k, :, :],
                        start=(k == 0),
                        stop=(k == KT - 1),
                    )
                    if k == KT - 1:
                        inst.then_inc(s_mm, 1)

        @block.vector
        def _(vector):
            if not with_clears:
                vector.memset(scr_sb[:], 0).then_inc(s_ws, 1)
            vector.wait_ge(s_cv, 16)  # cv loaded
            # s_mm counts: group 0 incs twice (its two half-groups), groups
            # 1-7 once each -> group s complete at s_mm >= s+2 (s>=1), 2 (s=0).
            # Each group is added in two 256-wide halves so the two store
            # rings can be fed independently; inc map:
            #   s0-s3 h1 -> s_dve_act 1..4 (scalar-ring stores)
            #   s4-s6 h1 -> s_dve_sync 1..3 (sync-ring stores)
            #   s7 h0 -> s_dve_act 5, s7 h1 -> s_dve_sync 4 (split tail)
            for s in range(ST):
                vector.wait_ge(s_mm, 2 if s == 0 else s + 2)
                for u in range(2):
                    inst = nc.vector.tensor_scalar_add(
                        out_sb[:, s, u * NQ:(u + 1) * NQ],
                        ps[:, s, u * NQ:(u + 1) * NQ],
                        cv_sb[:, s:s + 1],
                    )
                    if s == 7:
                        inst.then_inc(s_dve_act if u == 0 else s_dve_sync, 1)
                    elif u == 1:
                        inst.then_inc(s_dve_act if s <= 3 else s_dve_sync, 1)

    return nc


def _get_program():
    nc = _cache.get("nc")
    if nc is None:
        nc = _build_program()
        _cache["nc"] = nc
    return nc


def _prep_in_maps(x, idx, fbt, opt):
    bf = ml_dtypes.bfloat16
    in_maps = []
    for b in range(B):
        w = opt[idx[b]].reshape(F, D, O)                     # [F,D,O] f32
        wpack = w.transpose(1, 0, 2).reshape(KT, 128, ST, 128)  # [k,p,s,c]
        wp_host = np.ascontiguousarray(
            wpack.transpose(1, 2, 0, 3).reshape(128, KT * FO)
        ).astype(bf)                                         # [p, s*512+k*128+c]
        bias = fbt[idx[b]]                                   # [F,D]
        cvec = np.einsum("fd,fdo->fo", bias, w).reshape(FO).astype(np.float32)
        cv = np.ascontiguousarray(cvec.reshape(ST, 128).T)   # [128, ST]
        for h in range(2):
            xtT = x[b, h * NH:(h + 1) * NH, :].T             # [D, NH]
            xt_host = np.ascontiguousarray(
                xtT.reshape(KT, 128, 2, NQ).transpose(1, 0, 2, 3)
            ).astype(bf)                                     # [128, KT, 2, NQ]
            in_maps.append({"xt": xt_host, "wp": wp_host, "cv": cv})
    return in_maps


def _assemble(results):
    out = np.empty((B, N, F, O), dtype=np.float32)
    for c in range(8):
        b, h = divmod(c, 2)
        y = np.asarray(results[c]["y"]).astype(np.float32)   # [FO, NH] bf16
        out[b, h * NH:(h + 1) * NH] = y.reshape(F, O, NH).transpose(2, 0, 1)
    return out


def _run(x, idx, feature_bias_table, out_projection_table, **run_kwargs):
    from concourse.bass_utils import run_bass_kernel_spmd

    x = np.asarray(x, dtype=np.float32)
    idx = np.asarray(idx).astype(np.int64)
    fbt = np.asarray(feature_bias_table, dtype=np.float32)
    opt = np.asarray(out_projection_table, dtype=np.float32)

    nc = _get_program()
    in_maps = _prep_in_maps(x, idx, fbt, opt)
    res = run_bass_kernel_spmd(nc, in_maps, core_ids=list(range(8)), **run_kwargs)
    return _assemble(res.results), res


def kernel(x, idx, feature_bias_table, out_projection_table):
    out, _ = _run(x, idx, feature_bias_table, out_projection_table)
    return out
